# revision 3
# baseline (speedup 1.0000x reference)
"""Trainium2 Bass kernel for nn_Blur: 4x4 FIR depthwise blur with pad (2,1).

out[n,c,i,j] = sum_{a,b} K[a,b] * x[n,c, i+1-a, j+1-b]   (zero-padded)

Strategy (8 NeuronCores, pure data parallelism over the 8192 (n,c) slices):
  - fp16 end-to-end on device (host converts): halves HBM traffic vs fp32.
    Quantization error ~5e-4 relative, far under the 2e-2 gate.
  - w-parity interleaved layout: SBUF partition p = 64*(w%2) + h, free dim
    = (slice, w-block jb) with one zero w-block of left/right pad per slice.
    A single rhs column then carries BOTH w-parities of one w-block for all
    64 h rows, so each 128-wide contraction delivers up to 8 useful taps.
  - The 16-tap conv needs only THREE PSUM-accumulated matmuls (block shifts
    delta in {-1,0,+1} along the free dim) instead of four: lhsT_d[(jp_in,u),
    (jp_out,i)] = K[i-u+1, jp_out-jp_in+1-2d] (band in h, parity in w).
  - PE work: 12 x N=512 matmul-columns per 64-slice tile = 41us/core, which
    exactly matches the fp16 DMA span (~41us at the ~420 GB/s fabric rate);
    both engines stream stall-free.
  - Startup: weights go as one contiguous [128,384] DMA; two junk matmuls on
    a memset tile open the HAM clock gate (1.2 -> 2.4 GHz) with no DMA
    dependency; the first tile is split into 16-slice quarters so the first
    real matmul starts ~0.6us earlier.
  - Drain: the last tile stores per-PSUM-group, alternating both HWDGE
    rings, so the tail is one 128KB store instead of one 512KB store.
"""

import sys
import types

import numpy as np

import concourse.bacc as bacc
import concourse.mybir as mybir
from concourse.tile import TileContext
from concourse.bass_utils import run_bass_kernel_spmd


def _install_ntff_hook():
    """Best-effort shim: this image's antenv lacks axon_hooks, which the
    trace=True path of run_bass_kernel_spmd imports. Harmless if unused."""
    if "antenv.axon_hooks" in sys.modules:
        return
    try:
        sys.path.insert(0, "/root/.axon_site")
        from trn_agent_boot.trn_boot import _ntff_profile_via_ctypes

        hook = _ntff_profile_via_ctypes("/opt/axon/libaxon_pjrt.so")
        mod = types.ModuleType("antenv.axon_hooks")
        mod.get_axon_ntff_profile_hook = lambda: hook
        mod.set_axon_ntff_profile_hook = lambda h: None
        sys.modules["antenv.axon_hooks"] = mod
    except Exception:
        pass


_install_ntff_hook()

N_CORES = 8
B, C, H, W = 32, 256, 64, 64
NSLICES = B * C                      # 8192
SLICES_PER_CORE = NSLICES // N_CORES  # 1024
TILE_SLICES = 64                     # slices per full SBUF tile
JB = W // 2                          # 32 w-blocks of 2 per slice
JBP = JB + 2                         # +1 zero block left, +1 right
GQ = 16                              # slices per PSUM group (N = 16*32 = 512)
F16 = mybir.dt.float16
F32 = mybir.dt.float32

_NC_CACHE = {}


def _build_wmat(K: np.ndarray) -> np.ndarray:
    """[128, 3*128] fp16: contiguous lhsT stack for block shifts d=0,-1,+1."""
    K = np.asarray(K, np.float32)
    wmat = np.zeros((3, 128, 128), np.float32)
    for di, d in enumerate((0, -1, 1)):
        L = wmat[di]
        for jpi in range(2):
            for jpo in range(2):
                b = jpo - jpi + 1 - 2 * d
                if not (0 <= b < 4):
                    continue
                for i in range(H):
                    for a in range(4):
                        u = i + 1 - a
                        if 0 <= u < H:
                            L[64 * jpi + u, 64 * jpo + i] += K[a, b]
    # [d, k, m] -> [k, (d m)] so the DMA is one contiguous run per partition
    return np.ascontiguousarray(
        wmat.transpose(1, 0, 2).reshape(128, 3 * 128)
    ).astype(np.float16)


WARMUP_MMS = 2


def _build_nc(slices_per_core: int = SLICES_PER_CORE):
    ntiles = slices_per_core // TILE_SLICES
    nc = bacc.Bacc("TRN2", target_bir_lowering=False, debug=False)
    # DRAM layouts are the SBUF tile layouts (host pre-/post-permutes):
    #   x: [tile, p=(jp h), (s jbp)]  with jbp = [pad, 32 data blocks, pad]
    x = nc.dram_tensor(
        "x", [ntiles, 128, TILE_SLICES * JBP], F16, kind="ExternalInput"
    ).ap()
    wm = nc.dram_tensor("w", [128, 3 * 128], F16, kind="ExternalInput").ap()
    y = nc.dram_tensor(
        "y", [ntiles, 128, TILE_SLICES * JB], F16, kind="ExternalOutput"
    ).ap()
    # sink for the PE warm-up matmuls (kept alive so DCE can't drop them)
    warm_out = nc.dram_tensor("warm", [128, 4], F32, kind="ExternalOutput").ap()

    # chunk = (dram tile, slice offset in tile, n slices). The first tile is
    # quartered so the first matmul starts as soon as 139KB (not 557KB) lands;
    # the last is quartered on the STORE side only (per-group stores).
    chunks = [(0, so, GQ) for so in range(0, TILE_SLICES, GQ)]
    chunks += [(t, 0, TILE_SLICES) for t in range(1, ntiles)]

    with TileContext(nc) as tc:
        with (
            tc.tile_pool(name="wpool", bufs=1) as wpool,
            tc.tile_pool(name="xqpool", bufs=4) as xqpool,
            tc.tile_pool(name="xpool", bufs=6) as xpool,
            tc.tile_pool(name="oqpool", bufs=4) as oqpool,
            tc.tile_pool(name="opool", bufs=6) as opool,
            tc.tile_pool(name="pspool", bufs=8, space="PSUM") as pspool,
        ):
            wsb = wpool.tile([128, 3, 128], F16, name="wsb")
            nc.sync.dma_start(wsb[:], wm)

            # HAM warm-up with zero DMA dependency: junk matmuls on a memset
            # tile get the PE clock gate opening (1.2 -> 2.4 GHz) while the
            # weights and first input quarter are still in flight.
            wjunk = wpool.tile([128, 512], F16, name="wjunk")
            nc.vector.memset(wjunk[:], 0.0)
            wscratch = wpool.tile([128, 4], F32, name="wscratch")
            wps = pspool.tile([128, 512], F32, name="wps", tag="ps")
            for r in range(WARMUP_MMS):
                nc.tensor.matmul(
                    wps[:],
                    wjunk[:, 0:128],
                    wjunk[:],
                    start=(r == 0),
                    stop=(r == WARMUP_MMS - 1),
                )
            nc.vector.tensor_copy(wscratch[:], wps[:, 0:4])
            nc.scalar.dma_start(warm_out, wscratch[:])

            ncopy = 0
            for ci, (dt, so, ns) in enumerate(chunks):
                ng = ns // GQ
                qpools = (xqpool, oqpool) if ns == GQ else (xpool, opool)
                xt = qpools[0].tile([128, ns, JBP], F16, name="xt")
                nc.sync.dma_start(
                    xt[:], x[dt][:, so * JBP : (so + ns) * JBP]
                )

                ot = qpools[1].tile([128, ns, JB], F16, name="ot")
                pss = [
                    pspool.tile([128, GQ * JB], F32, name="ps")
                    for _ in range(ng)
                ]
                # d-outer loop: one stationary load per pass, each streaming
                # ng x 512 columns before the next LDWEIGHTS.
                for di, d in enumerate((0, -1, 1)):
                    for q in range(ng):
                        nc.tensor.matmul(
                            pss[q][:],
                            wsb[:, di, :],
                            xt[:, GQ * q : GQ * (q + 1), 1 + d : 1 + d + JB],
                            start=(di == 0),
                            stop=(di == 2),
                        )
                last_chunk = ci == len(chunks) - 1
                for q in range(ng):
                    # alternate copy engine: DVE and ACT share the load
                    dst = ot[:, GQ * q : GQ * (q + 1), :]
                    if ncopy % 2 == 0:
                        nc.vector.tensor_copy(dst, pss[q][:])
                    else:
                        nc.scalar.copy(dst, pss[q][:])
                    ncopy += 1
                    if last_chunk:
                        # drain per-group across BOTH rings (loads are all
                        # issued by now, so no head-of-line risk on sync)
                        store_eng = nc.sync if q % 2 == 0 else nc.scalar
                        store_eng.dma_start(
                            y[dt][
                                :,
                                (so + GQ * q) * JB : (so + GQ * (q + 1)) * JB,
                            ],
                            dst,
                        )
                if not last_chunk:
                    # single store per chunk on the ACT ring: never blocks
                    # the SP ring that feeds loads
                    nc.scalar.dma_start(
                        y[dt][:, so * JB : (so + ns) * JB], ot[:]
                    )

    nc.compile()
    return nc


def get_nc(slices_per_core: int = SLICES_PER_CORE):
    if slices_per_core not in _NC_CACHE:
        _NC_CACHE[slices_per_core] = _build_nc(slices_per_core)
    return _NC_CACHE[slices_per_core]


def _pack_input(xs: np.ndarray) -> np.ndarray:
    """[S, H, W] fp16 -> [S/64, 128, 64*JBP] in the SBUF tile layout."""
    s = xs.shape[0]
    ntiles = s // TILE_SLICES
    # [s, jp, h, jbp] with jbp zero-padded on both block ends
    v = np.zeros((s, 2, H, JBP), np.float16)
    v[:, 0, :, 1 : 1 + JB] = xs[:, :, 0::2]
    v[:, 1, :, 1 : 1 + JB] = xs[:, :, 1::2]
    # (t, s, jp, h, jbp) -> (t, jp, h, s, jbp)
    v = v.reshape(ntiles, TILE_SLICES, 2, H, JBP).transpose(0, 2, 3, 1, 4)
    return np.ascontiguousarray(v.reshape(ntiles, 128, TILE_SLICES * JBP))


def _unpack_output(yp: np.ndarray) -> np.ndarray:
    """[S/64, 128, 64*JB] fp16 -> [S, H, W] fp16."""
    ntiles = yp.shape[0]
    v = yp.reshape(ntiles, 2, H, TILE_SLICES, JB)        # [t, jp, i, s, jb]
    out = np.empty((ntiles, TILE_SLICES, H, W), np.float16)
    out[:, :, :, 0::2] = v[:, 0].transpose(0, 2, 1, 3)
    out[:, :, :, 1::2] = v[:, 1].transpose(0, 2, 1, 3)
    return out.reshape(ntiles * TILE_SLICES, H, W)


def kernel(x: np.ndarray, kernel: np.ndarray, _trace: bool = False, **_tkw):
    xh = np.asarray(x).astype(np.float16)
    wmat = _build_wmat(kernel)
    b, c, h, w = x.shape
    xs = xh.reshape(b * c, h, w)
    spc = (b * c) // N_CORES
    nc = get_nc(spc)
    in_maps = [
        {"x": _pack_input(xs[k * spc : (k + 1) * spc]), "w": wmat}
        for k in range(N_CORES)
    ]
    res = run_bass_kernel_spmd(
        nc, in_maps, list(range(N_CORES)), trace=_trace, **_tkw
    )
    out = np.concatenate(
        [_unpack_output(res.results[k]["y"]) for k in range(N_CORES)], axis=0
    )
    result = out.reshape(b, c, h, w).astype(np.float32)
    if _trace:
        return result, res
    return result


# revision 6
# speedup vs baseline: 1.0380x; 1.0380x over previous
"""Trainium2 Bass kernel for nn_Blur: 4x4 FIR depthwise blur with pad (2,1).

out[n,c,i,j] = sum_{a,b} K[a,b] * x[n,c, i+1-a, j+1-b]   (zero-padded)

Strategy (8 NeuronCores, pure data parallelism over the 8192 (n,c) slices):
  - fp16 end-to-end on device (host converts): halves HBM traffic vs fp32.
    Quantization error ~5e-4 relative, far under the 2e-2 gate.
  - w-parity interleaved layout: SBUF partition p = 64*(w%2) + h, free dim
    = (slice, w-block jb) with one zero w-block of left/right pad per slice.
    A single rhs column then carries BOTH w-parities of one w-block for all
    64 h rows, so each 128-wide contraction delivers up to 8 useful taps.
  - The 16-tap conv needs only THREE PSUM-accumulated matmuls (block shifts
    delta in {-1,0,+1} along the free dim) instead of four: lhsT_d[(jp_in,u),
    (jp_out,i)] = K[i-u+1, jp_out-jp_in+1-2d] (band in h, parity in w).
  - PE work: 12 x N=512 matmul-columns per 64-slice tile = 41us/core, which
    exactly matches the fp16 DMA span (~41us at the ~420 GB/s fabric rate);
    both engines stream stall-free.
  - Startup: weights go as one contiguous [128,384] DMA; two junk matmuls on
    a memset tile open the HAM clock gate (1.2 -> 2.4 GHz) with no DMA
    dependency; the first tile is split into 16-slice quarters so the first
    real matmul starts ~0.6us earlier.
  - Drain: the last tile stores per-PSUM-group, alternating both HWDGE
    rings, so the tail is one 128KB store instead of one 512KB store.
"""

import sys
import types

import numpy as np

import concourse.bacc as bacc
import concourse.mybir as mybir
from concourse.tile import TileContext
from concourse.bass_utils import run_bass_kernel_spmd


def _install_ntff_hook():
    """Best-effort shim: this image's antenv lacks axon_hooks, which the
    trace=True path of run_bass_kernel_spmd imports. Harmless if unused."""
    if "antenv.axon_hooks" in sys.modules:
        return
    try:
        sys.path.insert(0, "/root/.axon_site")
        from trn_agent_boot.trn_boot import _ntff_profile_via_ctypes

        hook = _ntff_profile_via_ctypes("/opt/axon/libaxon_pjrt.so")
        mod = types.ModuleType("antenv.axon_hooks")
        mod.get_axon_ntff_profile_hook = lambda: hook
        mod.set_axon_ntff_profile_hook = lambda h: None
        sys.modules["antenv.axon_hooks"] = mod
    except Exception:
        pass


_install_ntff_hook()

N_CORES = 8
B, C, H, W = 32, 256, 64, 64
NSLICES = B * C                      # 8192
SLICES_PER_CORE = NSLICES // N_CORES  # 1024
TILE_SLICES = 64                     # slices per full SBUF tile
JB = W // 2                          # 32 w-blocks of 2 per slice
JBP = JB + 2                         # +1 zero block left, +1 right
GQ = 16                              # slices per PSUM group (N = 16*32 = 512)
F16 = mybir.dt.float16
F32 = mybir.dt.float32

_NC_CACHE = {}


def _build_wmat(K: np.ndarray) -> np.ndarray:
    """[128, 3*128] fp16: contiguous lhsT stack for block shifts d=0,-1,+1."""
    K = np.asarray(K, np.float32)
    wmat = np.zeros((3, 128, 128), np.float32)
    for di, d in enumerate((0, -1, 1)):
        L = wmat[di]
        for jpi in range(2):
            for jpo in range(2):
                b = jpo - jpi + 1 - 2 * d
                if not (0 <= b < 4):
                    continue
                for i in range(H):
                    for a in range(4):
                        u = i + 1 - a
                        if 0 <= u < H:
                            L[64 * jpi + u, 64 * jpo + i] += K[a, b]
    # [d, k, m] -> [k, (d m)] so the DMA is one contiguous run per partition
    return np.ascontiguousarray(
        wmat.transpose(1, 0, 2).reshape(128, 3 * 128)
    ).astype(np.float16)


WARMUP_MMS = 5


def _build_nc(slices_per_core: int = SLICES_PER_CORE):
    ntiles = slices_per_core // TILE_SLICES
    nc = bacc.Bacc("TRN2", target_bir_lowering=False, debug=False)
    # DRAM layouts are the SBUF tile layouts (host pre-/post-permutes):
    #   x: [tile, p=(jp h), (s jbp)]  with jbp = [pad, 32 data blocks, pad]
    x = nc.dram_tensor(
        "x", [ntiles, 128, TILE_SLICES * JBP], F16, kind="ExternalInput"
    ).ap()
    wm = nc.dram_tensor("w", [128, 3 * 128], F16, kind="ExternalInput").ap()
    y = nc.dram_tensor(
        "y", [ntiles, 128, TILE_SLICES * JB], F16, kind="ExternalOutput"
    ).ap()
    # sink for the PE warm-up matmuls (kept alive so DCE can't drop them)
    warm_out = nc.dram_tensor("warm", [128, 4], F32, kind="ExternalOutput").ap()

    # Full 64-slice chunks: every extra DMA costs ~600ns of serial issue
    # time on the issuing engine plus ~1.7us completion latency, so fewer,
    # bigger transfers win (v3 post-mortem: quartering the first tile let
    # the PE outrun the load pipeline and stall).
    chunks = [(t, 0, TILE_SLICES) for t in range(ntiles)]

    with TileContext(nc) as tc:
        with (
            tc.tile_pool(name="wpool", bufs=1) as wpool,
            tc.tile_pool(name="xpool", bufs=8) as xpool,
            tc.tile_pool(name="opool", bufs=6) as opool,
            tc.tile_pool(name="pspool", bufs=8, space="PSUM") as pspool,
        ):
            wsb = wpool.tile([128, 3, 128], F16, name="wsb")
            nc.sync.dma_start(wsb[:], wm)

            # HAM warm-up with zero DMA dependency: junk matmuls on a memset
            # tile get the PE clock gate opening (1.2 -> 2.4 GHz) while the
            # weights and first input quarter are still in flight.
            wjunk = wpool.tile([128, 512], F16, name="wjunk")
            nc.vector.memset(wjunk[:], 0.0)
            wscratch = wpool.tile([128, 4], F32, name="wscratch")
            wps = pspool.tile([128, 512], F32, name="wps", tag="ps")
            for r in range(WARMUP_MMS):
                nc.tensor.matmul(
                    wps[:],
                    wjunk[:, 0:128],
                    wjunk[:],
                    start=(r == 0),
                    stop=(r == WARMUP_MMS - 1),
                )
            nc.vector.tensor_copy(wscratch[:], wps[:, 0:4])
            nc.scalar.dma_start(warm_out, wscratch[:])

            ncopy = 0
            for ci, (dt, so, ns) in enumerate(chunks):
                ng = ns // GQ
                xt = xpool.tile([128, ns, JBP], F16, name="xt")
                nc.sync.dma_start(
                    xt[:], x[dt][:, so * JBP : (so + ns) * JBP]
                )

                ot = opool.tile([128, ns, JB], F16, name="ot")
                pss = [
                    pspool.tile([128, GQ * JB], F32, name="ps")
                    for _ in range(ng)
                ]
                # d-outer loop: one stationary load per pass, each streaming
                # ng x 512 columns before the next LDWEIGHTS.
                for di, d in enumerate((0, -1, 1)):
                    for q in range(ng):
                        nc.tensor.matmul(
                            pss[q][:],
                            wsb[:, di, :],
                            xt[:, GQ * q : GQ * (q + 1), 1 + d : 1 + d + JB],
                            start=(di == 0),
                            stop=(di == 2),
                        )
                last_chunk = ci == len(chunks) - 1
                for q in range(ng):
                    # alternate copy engine: DVE and ACT share the load
                    dst = ot[:, GQ * q : GQ * (q + 1), :]
                    if ncopy % 2 == 0:
                        nc.vector.tensor_copy(dst, pss[q][:])
                    else:
                        nc.scalar.copy(dst, pss[q][:])
                    ncopy += 1
                    if last_chunk:
                        # drain per-group across BOTH rings (loads are all
                        # issued by now, so no head-of-line risk on sync)
                        store_eng = nc.sync if q % 2 == 0 else nc.scalar
                        store_eng.dma_start(
                            y[dt][
                                :,
                                (so + GQ * q) * JB : (so + GQ * (q + 1)) * JB,
                            ],
                            dst,
                        )
                if not last_chunk:
                    # single store per chunk on the ACT ring: never blocks
                    # the SP ring that feeds loads
                    nc.scalar.dma_start(
                        y[dt][:, so * JB : (so + ns) * JB], ot[:]
                    )

    nc.compile()
    return nc


def get_nc(slices_per_core: int = SLICES_PER_CORE):
    if slices_per_core not in _NC_CACHE:
        _NC_CACHE[slices_per_core] = _build_nc(slices_per_core)
    return _NC_CACHE[slices_per_core]


def _pack_input(xs: np.ndarray) -> np.ndarray:
    """[S, H, W] fp16 -> [S/64, 128, 64*JBP] in the SBUF tile layout."""
    s = xs.shape[0]
    ntiles = s // TILE_SLICES
    # [s, jp, h, jbp] with jbp zero-padded on both block ends
    v = np.zeros((s, 2, H, JBP), np.float16)
    v[:, 0, :, 1 : 1 + JB] = xs[:, :, 0::2]
    v[:, 1, :, 1 : 1 + JB] = xs[:, :, 1::2]
    # (t, s, jp, h, jbp) -> (t, jp, h, s, jbp)
    v = v.reshape(ntiles, TILE_SLICES, 2, H, JBP).transpose(0, 2, 3, 1, 4)
    return np.ascontiguousarray(v.reshape(ntiles, 128, TILE_SLICES * JBP))


def _unpack_output(yp: np.ndarray) -> np.ndarray:
    """[S/64, 128, 64*JB] fp16 -> [S, H, W] fp16."""
    ntiles = yp.shape[0]
    v = yp.reshape(ntiles, 2, H, TILE_SLICES, JB)        # [t, jp, i, s, jb]
    out = np.empty((ntiles, TILE_SLICES, H, W), np.float16)
    out[:, :, :, 0::2] = v[:, 0].transpose(0, 2, 1, 3)
    out[:, :, :, 1::2] = v[:, 1].transpose(0, 2, 1, 3)
    return out.reshape(ntiles * TILE_SLICES, H, W)


def kernel(x: np.ndarray, kernel: np.ndarray, _trace: bool = False, **_tkw):
    xh = np.asarray(x).astype(np.float16)
    wmat = _build_wmat(kernel)
    b, c, h, w = x.shape
    xs = xh.reshape(b * c, h, w)
    spc = (b * c) // N_CORES
    nc = get_nc(spc)
    in_maps = [
        {"x": _pack_input(xs[k * spc : (k + 1) * spc]), "w": wmat}
        for k in range(N_CORES)
    ]
    res = run_bass_kernel_spmd(
        nc, in_maps, list(range(N_CORES)), trace=_trace, **_tkw
    )
    out = np.concatenate(
        [_unpack_output(res.results[k]["y"]) for k in range(N_CORES)], axis=0
    )
    result = out.reshape(b, c, h, w).astype(np.float32)
    if _trace:
        return result, res
    return result
